# revision 18
# baseline (speedup 1.0000x reference)
"""Multi-head self-attention (B=2, N=2048, C=1024, H=16) on 8 TRN2 NeuronCores.

Sharding: data-parallel over batch (2) x tensor-parallel over heads (16/4=4).
Core c handles batch b=c//4 and heads [4*(c%4), 4*(c%4)+4).

Design: the kernel is a ridge between the scalar engine (128 exp activations
of [128,1024] = ~142us, the softmax) and the tensor engine (~138us of
matmuls). The schedule keeps the scalar engine saturated from ~13us:

  - Inputs are cast to fp16 AND laid out chunk-major on the host so every
    DMA lands with 2-8KB contiguous runs (descriptor-rate matters): weight
    groups on the scalar-engine HWDGE queue, x^T in four 1MB chunks on the
    sync queue, in need-order. A burst of throwaway matmuls during the DMA
    wait lifts the PE HAM clock gate to 2.4GHz before real work arrives.
  - Blocks interleave head pairs per query chunk ((ic0,p0),(ic0,p1),
    (ic1,p0)...) so each chunk's out-projection can run mid-kernel.
  - Per slot (= one key tile): scores matmul pair (row-group packed heads)
    -> exp (scalar engine) -> budgeted fillers (QKV projection half-groups,
    V tile half-groups, out-projection units from an EDF queue) -> the AV
    matmul LAG=6 slots behind (pt pool bufs=8).
  - AV uses the ones-augmented V trick ([V|1]^T @ P^T) so softmax sums fall
    out of the matmul; normalization = DMA sums row -> Newton reciprocal ->
    gpsimd partition broadcast -> DVE multiply (odd head shifts via DMA).
  - y is written as fp16 partials, summed on the host with b_out.

PSUM: scores 2x[128,1024] (4 banks) + shared qkv/out/warm tag 2x[128,512]
(2 banks) + 2 AV accumulators [65,512] (2 banks) = 8 banks exactly.
"""

import contextlib
from collections import deque

import numpy as np

import concourse.bass as bass
import concourse.bacc as bacc
import concourse.tile as tile
from concourse import library_config, mybir
from concourse.bass_utils import run_bass_kernel_spmd

B, NSEQ, CDIM, NHEADS, HD = 2, 2048, 1024, 16, 64
NH = 4          # heads per core
NCORES = 8
F32 = mybir.dt.float32
F16 = mybir.dt.float16
EXP = mybir.ActivationFunctionType.Exp
SCALE = HD ** -0.5
LAG = 6         # AV matmuls trail the exp by this many slots


def build_program():
    nc = bacc.Bacc("TRN2", target_bir_lowering=False, debug=False)

    # chunk-major host layouts (see make_in_maps)
    xT = nc.dram_tensor("xT", [4, 128, 8, 512], F16, kind="ExternalInput").ap()
    wpk = nc.dram_tensor("wpk", [4, 128, 8, 128], F16, kind="ExternalInput").ap()
    wv = nc.dram_tensor("wv", [128, 8, 2 * HD * NH // 2], F16, kind="ExternalInput").ap()
    wout = nc.dram_tensor("wout", [128, 2, CDIM], F16, kind="ExternalInput").ap()
    y = nc.dram_tensor("y", [NSEQ, CDIM], F16, kind="ExternalOutput").ap()

    with tile.TileContext(nc) as tc:
        emit(nc, tc, xT, wpk, wv, wout, y)

    nc.compile()
    return nc


def emit(nc, tc, xT, wpk, wv, wout, y):
    ctx = contextlib.ExitStack()
    with ctx:
        const = ctx.enter_context(tc.tile_pool(name="const", bufs=1))

        # ---- persistent SBUF tensors (fp16, DMA'd without staging) ----
        wf_sb = [const.tile([128, 8, 128], F16, name=f"wf{i}")
                 for i in range(4)]                          # q01,q23,k01,k23
        wv_sb = const.tile([128, 8, NH * HD], F16)           # [p, ct, 256]
        wout_sb = const.tile([128, 2, CDIM], F16)            # [p, ktile, 1024]
        xc = [const.tile([128, 8, 512], F16, name=f"xc{i}")
              for i in range(4)]                             # x^T chunks
        qk_sb = const.tile([128, 4, NSEQ], F16)              # q01,q23,k01,k23
        v_aug = const.tile([128, 16, NH, HD + 1], F16)       # [p, nt, head, V|1]
        o_sb = const.tile([128, 2, NSEQ], F16)               # normalized O^T

        nc.gpsimd.load_library(library_config.attn)
        nc.vector.memset(v_aug[:, :, :, HD:HD + 1], 1.0)

        with tc.tile_pool(name="pP", bufs=LAG + 2) as pP, \
             tc.tile_pool(name="oup", bufs=2) as oup, \
             tc.tile_pool(name="stat", bufs=2) as stat, \
             tc.tile_pool(name="rbc", bufs=4) as rbc, \
             tc.tile_pool(name="shf", bufs=2) as shf, \
             tc.tile_pool(name="yb", bufs=3) as yb, \
             tc.tile_pool(name="psm", bufs=1, space="PSUM") as psm:

            # ---- DMA schedule: ONE queue, strict need-order (the HWDGE
            # queues share one AXI port, so two active queues just halve
            # each other's rate; in-order single-queue is optimal) ----
            nc.sync.dma_start(xc[0], xT[0])
            nc.sync.dma_start(wf_sb[2], wpk[2])
            nc.sync.dma_start(wf_sb[0], wpk[0])
            nc.sync.dma_start(wv_sb, wv)
            nc.sync.dma_start(xc[1], xT[1])
            nc.sync.dma_start(wf_sb[3], wpk[3])
            nc.sync.dma_start(xc[2], xT[2])
            nc.sync.dma_start(wf_sb[1], wpk[1])
            nc.sync.dma_start(xc[3], xT[3])
            nc.sync.dma_start(wout_sb, wout)

            # ---- HAM warm-up: throwaway matmuls while the DMAs land ----
            warm = psm.tile([128, 512], F32, tag="mm", bufs=2, name="warm")
            for _ in range(8):
                nc.tensor.matmul(warm, qk_sb[:, 0, 0:128], qk_sb[:, 0, 0:512],
                                 start=True, stop=True)

            # ---- QKV projection pieces (emitted as half-groups) ----
            live_qk = {}

            def qk_half(ft, ic, half):
                # Q^T/K^T for 2 heads: [128 d, 512 seq] accumulated over 8
                # c-tiles; half 0 = ct 0-3, half 1 = ct 4-7 + cast to SBUF.
                if half == 0:
                    live_qk[(ft, ic)] = psm.tile([128, 512], F32, tag="mm",
                                                 bufs=2, name="psqk")
                ps = live_qk[(ft, ic)]
                for ct in range(4 * half, 4 * half + 4):
                    nc.tensor.matmul(
                        ps,
                        wf_sb[ft][:, ct, :],
                        xc[ic][:, ct, :],
                        start=(ct == 0), stop=(ct == 7),
                    )
                if half == 1:
                    nc.vector.tensor_copy(qk_sb[:, ft, ic * 512:(ic + 1) * 512], ps)
                    del live_qk[(ft, ic)]

            live_v = {}

            def v_half(nt, half):
                # V for all 4 heads at seq tile nt: [128 seq, 256] over 8
                # c-tiles; half 1 also scatters into v_aug's [V|1] layout.
                if half == 0:
                    live_v[nt] = psm.tile([128, NH * HD], F32, tag="mm",
                                          bufs=2, name="psvp")
                ps = live_v[nt]
                ix, nw = nt // 4, nt % 4
                for ct in range(4 * half, 4 * half + 4):
                    nc.tensor.matmul(
                        ps,
                        xc[ix][:, ct, nw * 128:(nw + 1) * 128],
                        wv_sb[:, ct, :],
                        start=(ct == 0), stop=(ct == 7),
                    )
                if half == 1:
                    for h in range(NH):
                        nc.vector.tensor_copy(
                            v_aug[:, nt, h, 0:HD], ps[:, h * HD:(h + 1) * HD])
                    del live_v[nt]

            def y_unit(it, fc, tag="mm", eng=0):
                # y[it*128:, fc*512:] = O_norm^T.T @ W_out, fp16 out to DRAM.
                psy = psm.tile([128, 512], F32, tag=tag, bufs=2, name="pyt")
                for pp in range(2):
                    nc.tensor.matmul(
                        psy,
                        o_sb[:, pp, it * 128:(it + 1) * 128],
                        wout_sb[:, pp, fc * 512:(fc + 1) * 512],
                        start=(pp == 0), stop=(pp == 1),
                    )
                y_sb = yb.tile([128, 512], F16, tag="ysb", name="ysbt")
                if eng:  # epilogue only: ACT engine is free, split the casts
                    nc.scalar.copy(y_sb, psy)
                else:
                    nc.vector.tensor_copy(y_sb, psy)
                nc.sync.dma_start(
                    y[it * 128:(it + 1) * 128, fc * 512:(fc + 1) * 512], y_sb)

            # ---- attention pieces ----
            live_po = {}

            def scores_act(p, ic, jt):
                ps = psm.tile([128, 1024], F32, tag="sb", bufs=2, name="pss")
                i0 = ic * 512
                for e in range(2):  # heads 2p, 2p+1 packed into PE row groups
                    pb = 64 * e
                    nc.tensor.matmul(
                        ps[:, e * 512:(e + 1) * 512],
                        qk_sb[pb:pb + 64, 2 + p, jt * 128:(jt + 1) * 128],
                        qk_sb[pb:pb + 64, p, i0:i0 + 512],
                        start=True, stop=True,
                        tile_position=(pb, 0),
                    )
                pt = pP.tile([128, 1024], F16, tag="p")
                nc.scalar.activation(pt, ps, EXP, scale=SCALE)
                return pt

            def av(p, ic, jt, pt):
                if jt == 0:
                    live_po[(p, ic)] = [
                        psm.tile([HD + 1, 512], F32, tag=f"o{e}", bufs=1,
                                 name=f"po{e}") for e in range(2)]
                po = live_po[(p, ic)]
                for e in range(2):
                    nc.tensor.matmul(
                        po[e],
                        v_aug[:, jt, 2 * p + e, :],
                        pt[:, e * 512:(e + 1) * 512],
                        start=(jt == 0), stop=(jt == 15),
                    )

            def norm(p, ic):
                # copy O_aug out of PSUM (frees the po banks), reciprocal of
                # the sums row, partition broadcast, multiply into o_sb.
                po = live_po.pop((p, ic))
                i0 = ic * 512
                for e in range(2):
                    o_u = oup.tile([HD + 1, 512], F32, tag=f"ou{e}",
                                   name=f"ou{e}")
                    with tc.high_priority():
                        nc.vector.tensor_copy(o_u, po[e])
                    r0 = stat.tile([1, 512], F32, tag=f"r0{e}", name=f"r0{e}")
                    nc.sync.dma_start(r0, o_u[HD:HD + 1, :])
                    r1 = stat.tile([1, 512], F32, tag=f"r1{e}", name=f"r1{e}")
                    rs = stat.tile([1, 512], F32, tag=f"rs{e}", name=f"rs{e}")
                    nc.vector.reciprocal_approx_accurate(r1, r0, rs)
                    rb = rbc.tile([64, 512], F32, tag="rb")
                    nc.gpsimd.partition_broadcast(rb, r1)
                    if e == 0:
                        nc.vector.tensor_mul(
                            o_sb[0:64, p, i0:i0 + 512], o_u[0:64, :], rb)
                    else:
                        tmp = shf.tile([64, 512], F16, tag="tmp")
                        nc.vector.tensor_mul(tmp, o_u[0:64, :], rb)
                        nc.sync.dma_start(o_sb[64:128, p, i0:i0 + 512], tmp)

            def norm_epi(p, ic):
                # latency-optimized final normalization: both chains issued
                # eagerly so DVE/gpsimd/DMA pipeline, with throwaway matmuls
                # holding the PE HAM clock at 2.4GHz through the chain.
                po = live_po.pop((p, ic))
                i0 = ic * 512
                ous, r1s = [], []
                for e in range(2):
                    o_u = oup.tile([HD + 1, 512], F32, tag=f"ou{e}",
                                   name=f"oue{e}")
                    with tc.high_priority():
                        nc.vector.tensor_copy(o_u, po[e])
                    r0 = stat.tile([1, 512], F32, tag=f"r0{e}", name=f"r0e{e}")
                    nc.sync.dma_start(r0, o_u[HD:HD + 1, :])
                    r1 = stat.tile([1, 512], F32, tag=f"r1{e}", name=f"r1e{e}")
                    rs = stat.tile([1, 512], F32, tag=f"rs{e}", name=f"rse{e}")
                    nc.vector.reciprocal_approx_accurate(r1, r0, rs)
                    ous.append(o_u)
                    r1s.append(r1)
                wt = psm.tile([128, 512], F32, tag="sb", bufs=2, name="wt")
                for _ in range(16):
                    nc.tensor.matmul(wt, qk_sb[:, 0, 0:128],
                                     qk_sb[:, 0, 0:512], start=True, stop=True)
                for e in range(2):
                    rb = rbc.tile([64, 512], F32, tag="rb")
                    nc.gpsimd.partition_broadcast(rb, r1s[e])
                    if e == 0:
                        nc.vector.tensor_mul(
                            o_sb[0:64, p, i0:i0 + 512], ous[0][0:64, :], rb)
                    else:
                        tmp = shf.tile([64, 512], F16, tag="tmp")
                        nc.vector.tensor_mul(tmp, ous[1][0:64, :], rb)
                        nc.sync.dma_start(o_sb[64:128, p, i0:i0 + 512], tmp)

            # ---- EDF filler queue, drained on a per-slot slack budget ----
            # entry: [cost_us, min_slot, thunk]
            fq = deque()

            def push_qk(ft, ic):
                fq.append([0.9, 0, lambda: qk_half(ft, ic, 0)])
                fq.append([0.9, 0, lambda: qk_half(ft, ic, 1)])

            def push_v(nt):
                fq.append([0.9, 0, lambda: v_half(nt, 0)])
                fq.append([0.9, 0, lambda: v_half(nt, 1)])

            # deadline-ordered initial work (pair-interleaved block order):
            # k01 icN by slot 4N; v nt by slot nt+5 (AV lag 6); k23/q23 ic0
            # by slot 15; k23 icN by slot 16+4N; q01/q23 icN by slot 32N/+16.
            push_qk(2, 1)
            push_v(0)
            push_v(1)
            push_qk(2, 2)
            push_v(2)
            push_v(3)
            push_qk(3, 0)
            push_qk(1, 0)
            push_v(4)
            push_v(5)
            push_qk(2, 3)
            push_v(6)
            push_v(7)
            push_qk(3, 1)
            push_v(8)
            push_v(9)
            push_v(10)
            push_v(11)
            push_v(12)
            push_v(13)
            push_qk(3, 2)
            push_v(14)
            push_v(15)
            push_qk(3, 3)
            push_qk(0, 1)
            push_qk(1, 1)
            push_qk(0, 2)
            push_qk(1, 2)
            push_qk(0, 3)
            push_qk(1, 3)

            # ---- lead: K and Q for the first query block ----
            qk_half(2, 0, 0)
            qk_half(2, 0, 1)
            qk_half(0, 0, 0)
            qk_half(0, 0, 1)

            # ---- 128-slot pipeline, head pairs interleaved per chunk ----
            slots = [(p, ic, jt)
                     for ic in range(4) for p in range(2) for jt in range(16)]
            pending = deque()
            budget = 0.0

            def drain_av(target):
                while len(pending) > target:
                    ap, aic, ajt, apt = pending.popleft()
                    av(ap, aic, ajt, apt)
                    if ajt == 15:
                        if (ap, aic) == (1, 3):
                            norm_epi(ap, aic)
                        else:
                            norm(ap, aic)
                        if ap == 1 and aic < 3:
                            # out-projection for this query chunk, spaced
                            # one unit per ~2 slots once the norm lands
                            for k in range(8):
                                fq.append([0.9, cur_slot + 4,
                                           (lambda it=4 * aic + k // 2,
                                            fc=k % 2: y_unit(it, fc))])

            for s, (p, ic, jt) in enumerate(slots):
                cur_slot = s
                pt = scores_act(p, ic, jt)
                pending.append((p, ic, jt, pt))
                rate, cap = ((2.7, 3.0) if s < 6 else
                             (1.9, 2.2) if s < 22 else (0.5, 1.0))
                budget = min(budget + rate, cap)
                while fq and budget >= fq[0][0] and s >= fq[0][1]:
                    c, _, th = fq.popleft()
                    th()
                    budget -= c
                drain_av(LAG if s < 122 else max(0, LAG - (s - 121)))
            cur_slot = 128
            drain_av(0)
            for k in range(8):
                y_unit(12 + k // 2, k % 2, tag=("mm", "sb")[k % 2], eng=k % 2)
            while fq:  # anything the budget never drained (shouldn't happen)
                c, _, th = fq.popleft()
                th()


_NC = None


def _get_nc():
    global _NC
    if _NC is None:
        _NC = build_program()
    return _NC


def make_in_maps(x, w_qkv, w_out):
    x = np.asarray(x, dtype=np.float16)
    w_qkv = np.asarray(w_qkv, dtype=np.float16)
    w_out = np.asarray(w_out, dtype=np.float16)
    in_maps = []
    for c in range(NCORES):
        b, g = divmod(c, 4)
        f0 = g * NH * HD  # first feature col of this head group (256 wide)
        wq = w_qkv[:, f0:f0 + 256]
        wk = w_qkv[:, CDIM + f0:CDIM + f0 + 256]
        wv = w_qkv[:, 2 * CDIM + f0:2 * CDIM + f0 + 256]
        wqk = np.concatenate([wq, wk], axis=1)          # [1024, 512]
        xT = x[b].T                                     # [1024, 2048]
        in_maps.append({
            # [ic, p, t, n] — per-partition contiguous 8KB runs
            "xT": np.ascontiguousarray(
                xT.reshape(8, 128, 4, 512).transpose(2, 1, 0, 3)),
            # [ft, p, t, f] with ft = q01,q23,k01,k23
            "wpk": np.ascontiguousarray(
                wqk.reshape(8, 128, 4, 128).transpose(2, 1, 0, 3)),
            # [p, t, f]
            "wv": np.ascontiguousarray(
                wv.reshape(8, 128, 256).transpose(1, 0, 2)),
            # [p, kt, f]
            "wout": np.ascontiguousarray(
                w_out[f0:f0 + 256, :].reshape(2, 128, CDIM).transpose(1, 0, 2)),
        })
    return in_maps


def kernel(x, w_qkv, b_qkv, w_out, b_out, _trace=False):
    """Full inputs in, full (B, N, C) output out. b_qkv is all-zeros by the
    problem's input spec (fill: zeros); b_out is added on the host."""
    nc = _get_nc()
    in_maps = make_in_maps(x, w_qkv, w_out)
    res = run_bass_kernel_spmd(nc, in_maps, core_ids=list(range(NCORES)),
                               trace=_trace)
    out = np.zeros((B, NSEQ, CDIM), dtype=np.float32)
    for c in range(NCORES):
        out[c // 4] += np.asarray(res.results[c]["y"], dtype=np.float32)
    out += np.asarray(b_out, dtype=np.float32)
    if _trace:
        kernel.last_exec_time_ns = res.exec_time_ns
        kernel.last_results = res
    return out


# revision 19
# speedup vs baseline: 1.1790x; 1.1790x over previous
"""Multi-head self-attention (B=2, N=2048, C=1024, H=16) on 8 TRN2 NeuronCores.

Sharding: data-parallel over batch (2) x tensor-parallel over heads (16/4=4).
Core c handles batch b=c//4 and heads [4*(c%4), 4*(c%4)+4).

Design: the kernel is a ridge between the scalar engine (128 exp activations
of [128,1024] = ~142us, the softmax) and the tensor engine (~138us of
matmuls). The schedule keeps the scalar engine saturated from ~13us:

  - Inputs are cast to fp16 AND laid out chunk-major on the host so every
    DMA lands with 2-8KB contiguous runs (descriptor-rate matters): weight
    groups on the scalar-engine HWDGE queue, x^T in four 1MB chunks on the
    sync queue, in need-order. A burst of throwaway matmuls during the DMA
    wait lifts the PE HAM clock gate to 2.4GHz before real work arrives.
  - Blocks interleave head pairs per query chunk ((ic0,p0),(ic0,p1),
    (ic1,p0)...) so each chunk's out-projection can run mid-kernel.
  - Per slot (= one key tile): scores matmul pair (row-group packed heads)
    -> exp (scalar engine) -> budgeted fillers (QKV projection half-groups,
    V tile half-groups, out-projection units from an EDF queue) -> the AV
    matmul LAG=6 slots behind (pt pool bufs=8).
  - AV uses the ones-augmented V trick ([V|1]^T @ P^T) so softmax sums fall
    out of the matmul; normalization = DMA sums row -> Newton reciprocal ->
    gpsimd partition broadcast -> DVE multiply (odd head shifts via DMA).
  - y is written as fp16 partials, summed on the host with b_out.

PSUM: scores 2x[128,1024] (4 banks) + shared qkv/out/warm tag 2x[128,512]
(2 banks) + 2 AV accumulators [65,512] (2 banks) = 8 banks exactly.
"""

import contextlib
from collections import deque

import numpy as np

import concourse.bass as bass
import concourse.bacc as bacc
import concourse.tile as tile
from concourse import library_config, mybir
from concourse.bass_utils import run_bass_kernel_spmd

B, NSEQ, CDIM, NHEADS, HD = 2, 2048, 1024, 16, 64
NH = 4          # heads per core
NCORES = 8
F32 = mybir.dt.float32
F16 = mybir.dt.float16
EXP = mybir.ActivationFunctionType.Exp
SCALE = HD ** -0.5
LAG = 6         # AV matmuls trail the exp by this many slots


def build_program():
    nc = bacc.Bacc("TRN2", target_bir_lowering=False, debug=False)

    # chunk-major host layouts (see make_in_maps)
    xT = nc.dram_tensor("xT", [4, 128, 8, 512], F16, kind="ExternalInput").ap()
    wpk = nc.dram_tensor("wpk", [4, 128, 8, 128], F16, kind="ExternalInput").ap()
    wv = nc.dram_tensor("wv", [128, 8, 2 * HD * NH // 2], F16, kind="ExternalInput").ap()
    wout = nc.dram_tensor("wout", [128, 2, CDIM], F16, kind="ExternalInput").ap()
    y = nc.dram_tensor("y", [NSEQ, CDIM], F16, kind="ExternalOutput").ap()

    with tile.TileContext(nc) as tc:
        emit(nc, tc, xT, wpk, wv, wout, y)

    nc.compile()
    return nc


def emit(nc, tc, xT, wpk, wv, wout, y):
    ctx = contextlib.ExitStack()
    with ctx:
        const = ctx.enter_context(tc.tile_pool(name="const", bufs=1))

        # ---- persistent SBUF tensors (fp16, DMA'd without staging) ----
        wf_sb = [const.tile([128, 8, 128], F16, name=f"wf{i}")
                 for i in range(4)]                          # q01,q23,k01,k23
        wv_sb = const.tile([128, 8, NH * HD], F16)           # [p, ct, 256]
        wout_sb = const.tile([128, 2, CDIM], F16)            # [p, ktile, 1024]
        xc = [const.tile([128, 8, 512], F16, name=f"xc{i}")
              for i in range(4)]                             # x^T chunks
        qk_sb = const.tile([128, 4, NSEQ], F16)              # q01,q23,k01,k23
        v_aug = const.tile([128, 16, NH, HD + 1], F16)       # [p, nt, head, V|1]
        o_sb = const.tile([128, 2, NSEQ], F16)               # normalized O^T

        nc.gpsimd.load_library(library_config.attn)
        nc.vector.memset(v_aug[:, :, :, HD:HD + 1], 1.0)

        with tc.tile_pool(name="pP", bufs=LAG + 2) as pP, \
             tc.tile_pool(name="oup", bufs=2) as oup, \
             tc.tile_pool(name="stat", bufs=2) as stat, \
             tc.tile_pool(name="rbc", bufs=4) as rbc, \
             tc.tile_pool(name="shf", bufs=2) as shf, \
             tc.tile_pool(name="yb", bufs=3) as yb, \
             tc.tile_pool(name="psm", bufs=1, space="PSUM") as psm:

            # ---- DMA schedule: ONE queue, strict need-order (the HWDGE
            # queues share one AXI port, so two active queues just halve
            # each other's rate; in-order single-queue is optimal) ----
            nc.sync.dma_start(xc[0], xT[0])
            nc.sync.dma_start(wf_sb[2], wpk[2])
            nc.sync.dma_start(wf_sb[0], wpk[0])
            nc.sync.dma_start(wv_sb, wv)
            nc.sync.dma_start(xc[1], xT[1])
            nc.sync.dma_start(wf_sb[3], wpk[3])
            nc.sync.dma_start(xc[2], xT[2])
            nc.sync.dma_start(wf_sb[1], wpk[1])
            nc.sync.dma_start(xc[3], xT[3])
            nc.sync.dma_start(wout_sb, wout)

            # ---- HAM warm-up: throwaway matmuls while the DMAs land ----
            warm = psm.tile([128, 512], F32, tag="mm", bufs=2, name="warm")
            for _ in range(8):
                nc.tensor.matmul(warm, qk_sb[:, 0, 0:128], qk_sb[:, 0, 0:512],
                                 start=True, stop=True)

            # ---- QKV projection pieces (emitted as half-groups) ----
            live_qk = {}

            def qk_half(ft, ic, half):
                # Q^T/K^T for 2 heads: [128 d, 512 seq] accumulated over 8
                # c-tiles; half 0 = ct 0-3, half 1 = ct 4-7 + cast to SBUF.
                if half == 0:
                    live_qk[(ft, ic)] = psm.tile([128, 512], F32, tag="mm",
                                                 bufs=2, name="psqk")
                ps = live_qk[(ft, ic)]
                for ct in range(4 * half, 4 * half + 4):
                    nc.tensor.matmul(
                        ps,
                        wf_sb[ft][:, ct, :],
                        xc[ic][:, ct, :],
                        start=(ct == 0), stop=(ct == 7),
                    )
                if half == 1:
                    nc.vector.tensor_copy(qk_sb[:, ft, ic * 512:(ic + 1) * 512], ps)
                    del live_qk[(ft, ic)]

            live_v = {}

            def v_half(nt, half):
                # V for all 4 heads at seq tile nt: [128 seq, 256] over 8
                # c-tiles; half 1 also scatters into v_aug's [V|1] layout.
                if half == 0:
                    live_v[nt] = psm.tile([128, NH * HD], F32, tag="mm",
                                          bufs=2, name="psvp")
                ps = live_v[nt]
                ix, nw = nt // 4, nt % 4
                for ct in range(4 * half, 4 * half + 4):
                    nc.tensor.matmul(
                        ps,
                        xc[ix][:, ct, nw * 128:(nw + 1) * 128],
                        wv_sb[:, ct, :],
                        start=(ct == 0), stop=(ct == 7),
                    )
                if half == 1:
                    for h in range(NH):
                        nc.vector.tensor_copy(
                            v_aug[:, nt, h, 0:HD], ps[:, h * HD:(h + 1) * HD])
                    del live_v[nt]

            def y_unit(it, fc, tag="mm", eng=0):
                # y[it*128:, fc*512:] = O_norm^T.T @ W_out, fp16 out to DRAM.
                psy = psm.tile([128, 512], F32, tag=tag, bufs=2, name="pyt")
                for pp in range(2):
                    nc.tensor.matmul(
                        psy,
                        o_sb[:, pp, it * 128:(it + 1) * 128],
                        wout_sb[:, pp, fc * 512:(fc + 1) * 512],
                        start=(pp == 0), stop=(pp == 1),
                    )
                y_sb = yb.tile([128, 512], F16, tag="ysb", name="ysbt")
                if eng:  # epilogue only: ACT engine is free, split the casts
                    nc.scalar.copy(y_sb, psy)
                else:
                    nc.vector.tensor_copy(y_sb, psy)
                nc.sync.dma_start(
                    y[it * 128:(it + 1) * 128, fc * 512:(fc + 1) * 512], y_sb)

            # ---- attention pieces ----
            live_po = {}

            def scores_act(p, ic, jt):
                ps = psm.tile([128, 1024], F32, tag="sb", bufs=2, name="pss")
                i0 = ic * 512
                for e in range(2):  # heads 2p, 2p+1 packed into PE row groups
                    pb = 64 * e
                    nc.tensor.matmul(
                        ps[:, e * 512:(e + 1) * 512],
                        qk_sb[pb:pb + 64, 2 + p, jt * 128:(jt + 1) * 128],
                        qk_sb[pb:pb + 64, p, i0:i0 + 512],
                        start=True, stop=True,
                        tile_position=(pb, 0),
                    )
                pt = pP.tile([128, 1024], F16, tag="p")
                nc.scalar.activation(pt, ps, EXP, scale=SCALE)
                return pt

            def av(p, ic, jt, pt):
                if jt == 0:
                    live_po[(p, ic)] = [
                        psm.tile([HD + 1, 512], F32, tag=f"o{e}", bufs=1,
                                 name=f"po{e}") for e in range(2)]
                po = live_po[(p, ic)]
                for e in range(2):
                    nc.tensor.matmul(
                        po[e],
                        v_aug[:, jt, 2 * p + e, :],
                        pt[:, e * 512:(e + 1) * 512],
                        start=(jt == 0), stop=(jt == 15),
                    )

            def norm(p, ic):
                # copy O_aug out of PSUM (frees the po banks), reciprocal of
                # the sums row, partition broadcast, multiply into o_sb.
                po = live_po.pop((p, ic))
                i0 = ic * 512
                for e in range(2):
                    o_u = oup.tile([HD + 1, 512], F32, tag=f"ou{e}",
                                   name=f"ou{e}")
                    nc.vector.tensor_copy(o_u, po[e])
                    r0 = stat.tile([1, 512], F32, tag=f"r0{e}", name=f"r0{e}")
                    nc.sync.dma_start(r0, o_u[HD:HD + 1, :])
                    r1 = stat.tile([1, 512], F32, tag=f"r1{e}", name=f"r1{e}")
                    rs = stat.tile([1, 512], F32, tag=f"rs{e}", name=f"rs{e}")
                    nc.vector.reciprocal_approx_accurate(r1, r0, rs)
                    rb = rbc.tile([64, 512], F32, tag="rb")
                    nc.gpsimd.partition_broadcast(rb, r1)
                    if e == 0:
                        nc.vector.tensor_mul(
                            o_sb[0:64, p, i0:i0 + 512], o_u[0:64, :], rb)
                    else:
                        tmp = shf.tile([64, 512], F16, tag="tmp")
                        nc.vector.tensor_mul(tmp, o_u[0:64, :], rb)
                        nc.sync.dma_start(o_sb[64:128, p, i0:i0 + 512], tmp)

            def norm_epi(p, ic):
                # latency-optimized final normalization: both chains issued
                # eagerly so DVE/gpsimd/DMA pipeline, with throwaway matmuls
                # holding the PE HAM clock at 2.4GHz through the chain.
                po = live_po.pop((p, ic))
                i0 = ic * 512
                ous, r1s = [], []
                for e in range(2):
                    o_u = oup.tile([HD + 1, 512], F32, tag=f"ou{e}",
                                   name=f"oue{e}")
                    nc.vector.tensor_copy(o_u, po[e])
                    r0 = stat.tile([1, 512], F32, tag=f"r0{e}", name=f"r0e{e}")
                    nc.sync.dma_start(r0, o_u[HD:HD + 1, :])
                    r1 = stat.tile([1, 512], F32, tag=f"r1{e}", name=f"r1e{e}")
                    rs = stat.tile([1, 512], F32, tag=f"rs{e}", name=f"rse{e}")
                    nc.vector.reciprocal_approx_accurate(r1, r0, rs)
                    ous.append(o_u)
                    r1s.append(r1)
                wt = psm.tile([128, 512], F32, tag="sb", bufs=2, name="wt")
                for _ in range(16):
                    nc.tensor.matmul(wt, qk_sb[:, 0, 0:128],
                                     qk_sb[:, 0, 0:512], start=True, stop=True)
                for e in range(2):
                    rb = rbc.tile([64, 512], F32, tag="rb")
                    nc.gpsimd.partition_broadcast(rb, r1s[e])
                    if e == 0:
                        nc.vector.tensor_mul(
                            o_sb[0:64, p, i0:i0 + 512], ous[0][0:64, :], rb)
                    else:
                        tmp = shf.tile([64, 512], F16, tag="tmp")
                        nc.vector.tensor_mul(tmp, ous[1][0:64, :], rb)
                        nc.sync.dma_start(o_sb[64:128, p, i0:i0 + 512], tmp)

            # ---- EDF filler queue, drained on a per-slot slack budget ----
            # entry: [cost_us, min_slot, thunk]
            fq = deque()

            def push_qk(ft, ic):
                fq.append([0.9, 0, lambda: qk_half(ft, ic, 0)])
                fq.append([0.9, 0, lambda: qk_half(ft, ic, 1)])

            def push_v(nt):
                fq.append([0.9, 0, lambda: v_half(nt, 0)])
                fq.append([0.9, 0, lambda: v_half(nt, 1)])

            # deadline-ordered initial work (pair-interleaved block order):
            # k01 icN by slot 4N; v nt by slot nt+5 (AV lag 6); k23/q23 ic0
            # by slot 15; k23 icN by slot 16+4N; q01/q23 icN by slot 32N/+16.
            push_qk(2, 1)
            push_v(0)
            push_v(1)
            push_qk(2, 2)
            push_v(2)
            push_v(3)
            push_qk(3, 0)
            push_qk(1, 0)
            push_v(4)
            push_v(5)
            push_qk(2, 3)
            push_v(6)
            push_v(7)
            push_qk(3, 1)
            push_v(8)
            push_v(9)
            push_v(10)
            push_v(11)
            push_v(12)
            push_v(13)
            push_qk(3, 2)
            push_v(14)
            push_v(15)
            push_qk(3, 3)
            push_qk(0, 1)
            push_qk(1, 1)
            push_qk(0, 2)
            push_qk(1, 2)
            push_qk(0, 3)
            push_qk(1, 3)

            # ---- lead: K and Q for the first query block ----
            qk_half(2, 0, 0)
            qk_half(2, 0, 1)
            qk_half(0, 0, 0)
            qk_half(0, 0, 1)

            # ---- 128-slot pipeline, head pairs interleaved per chunk ----
            slots = [(p, ic, jt)
                     for ic in range(4) for p in range(2) for jt in range(16)]
            pending = deque()
            budget = 0.0

            def drain_av(target):
                while len(pending) > target:
                    ap, aic, ajt, apt = pending.popleft()
                    av(ap, aic, ajt, apt)
                    if ajt == 15:
                        if (ap, aic) == (1, 3):
                            norm_epi(ap, aic)
                        else:
                            norm(ap, aic)
                        if ap == 1 and aic < 3:
                            # out-projection for this query chunk, spaced
                            # one unit per ~2 slots once the norm lands
                            for k in range(8):
                                fq.append([0.9, cur_slot + 4,
                                           (lambda it=4 * aic + k // 2,
                                            fc=k % 2: y_unit(it, fc))])

            for s, (p, ic, jt) in enumerate(slots):
                cur_slot = s
                pt = scores_act(p, ic, jt)
                pending.append((p, ic, jt, pt))
                rate, cap = ((2.7, 3.0) if s < 6 else
                             (1.9, 2.2) if s < 22 else (0.5, 1.0))
                budget = min(budget + rate, cap)
                while fq and budget >= fq[0][0] and s >= fq[0][1]:
                    c, _, th = fq.popleft()
                    th()
                    budget -= c
                drain_av(LAG if s < 122 else max(0, LAG - (s - 121)))
            cur_slot = 128
            drain_av(0)
            for k in range(8):
                y_unit(12 + k // 2, k % 2, tag=("mm", "sb")[k % 2], eng=k % 2)
            while fq:  # anything the budget never drained (shouldn't happen)
                c, _, th = fq.popleft()
                th()


_NC = None


def _get_nc():
    global _NC
    if _NC is None:
        _NC = build_program()
    return _NC


def make_in_maps(x, w_qkv, w_out):
    x = np.asarray(x, dtype=np.float16)
    w_qkv = np.asarray(w_qkv, dtype=np.float16)
    w_out = np.asarray(w_out, dtype=np.float16)
    in_maps = []
    for c in range(NCORES):
        b, g = divmod(c, 4)
        f0 = g * NH * HD  # first feature col of this head group (256 wide)
        wq = w_qkv[:, f0:f0 + 256]
        wk = w_qkv[:, CDIM + f0:CDIM + f0 + 256]
        wv = w_qkv[:, 2 * CDIM + f0:2 * CDIM + f0 + 256]
        wqk = np.concatenate([wq, wk], axis=1)          # [1024, 512]
        xT = x[b].T                                     # [1024, 2048]
        in_maps.append({
            # [ic, p, t, n] — per-partition contiguous 8KB runs
            "xT": np.ascontiguousarray(
                xT.reshape(8, 128, 4, 512).transpose(2, 1, 0, 3)),
            # [ft, p, t, f] with ft = q01,q23,k01,k23
            "wpk": np.ascontiguousarray(
                wqk.reshape(8, 128, 4, 128).transpose(2, 1, 0, 3)),
            # [p, t, f]
            "wv": np.ascontiguousarray(
                wv.reshape(8, 128, 256).transpose(1, 0, 2)),
            # [p, kt, f]
            "wout": np.ascontiguousarray(
                w_out[f0:f0 + 256, :].reshape(2, 128, CDIM).transpose(1, 0, 2)),
        })
    return in_maps


def kernel(x, w_qkv, b_qkv, w_out, b_out, _trace=False):
    """Full inputs in, full (B, N, C) output out. b_qkv is all-zeros by the
    problem's input spec (fill: zeros); b_out is added on the host."""
    nc = _get_nc()
    in_maps = make_in_maps(x, w_qkv, w_out)
    res = run_bass_kernel_spmd(nc, in_maps, core_ids=list(range(NCORES)),
                               trace=_trace)
    out = np.zeros((B, NSEQ, CDIM), dtype=np.float32)
    for c in range(NCORES):
        out[c // 4] += np.asarray(res.results[c]["y"], dtype=np.float32)
    out += np.asarray(b_out, dtype=np.float32)
    if _trace:
        kernel.last_exec_time_ns = res.exec_time_ns
        kernel.last_results = res
    return out
